# revision 32
# baseline (speedup 1.0000x reference)
"""Cross-attention kernel for 8 Trainium2 NeuronCores.

Sharding: core c => batch b = c//4, head-group g = c%4 (3 of 12 heads, 192 dims).
Each core projects q/k/v for its heads, does softmax attention, and computes a
partial output projection (row-split Wo); host sums the 4 partials per batch.

Schedule (v2):
  - fill: packed-weight DMA + qT halves; q projections while kT/vT stream in.
  - pass1: per-group k projections + per-chunk v projections interleaved with
    h2 scores/exp/attn, so the Activation exp stream starts early and the PE
    stream stays dense while DMA feeds the rest of kT/vT.
  - pass2: h0+h1 scores/exp/attn (Activation-bound, PE has slack); h2 is
    normalized here on DVE+Pool, off the critical path.
  - tail: softmax denominators via DVE reciprocal_approx_fast (no Ln/Exp
    act-table loads), Pool partition-broadcast, DVE muls; Wo accumulates both
    contraction halves in PSUM (no SBUF staging / extra add).
  - mask compaction on host: only mask==1 key/value positions are shipped,
    zero-padded to a multiple of 128; padded rows have zero v and zero
    ones-column so they contribute 0 to numerator and denominator Z.
  - softmax without max-subtraction (scores*scale ~ N(0,1)); Z comes from a
    ones-column appended to v, so only the 64xN attention output is divided.
"""

import numpy as np

import concourse.bass as bass
import concourse.mybir as mybir
import concourse.tile as tile
from concourse import bacc
from concourse.bass_utils import run_bass_kernel_spmd

H = 12
D = 768
HD = 64
SCALE = HD ** -0.5
NQ = 1024
HL = 3            # heads per core
HWID = HL * HD    # 192 head dims per core
DC = D // 128     # 6 contraction chunks

f16 = mybir.dt.float16
f32 = mybir.dt.float32

_programs = {}
DEBUG = False


def _build(SP: int):
    NCH = SP // 128
    nc = bacc.Bacc("TRN2", target_bir_lowering=False, debug=False, num_devices=8)

    qT = nc.dram_tensor("qT", [D, NQ], f16, kind="ExternalInput")
    kT = nc.dram_tensor("kT", [D, SP], f16, kind="ExternalInput")
    vT = nc.dram_tensor("vT", [D, SP], f16, kind="ExternalInput")
    mv = nc.dram_tensor("mv", [SP], f16, kind="ExternalInput")
    wqkv = nc.dram_tensor("wqkv", [D, 3 * HWID], f16, kind="ExternalInput")
    wop = nc.dram_tensor("wop", [128, 1536], f16, kind="ExternalInput")
    out = nc.dram_tensor("out", [NQ, D], f16, kind="ExternalOutput")
    if DEBUG:
        dq0 = nc.dram_tensor("dq0", [128, NQ], f16, kind="ExternalOutput")
        dq1 = nc.dram_tensor("dq1", [64, NQ], f16, kind="ExternalOutput")
        dk0 = nc.dram_tensor("dk0", [128, SP], f16, kind="ExternalOutput")
        dk1 = nc.dram_tensor("dk1", [64, SP], f16, kind="ExternalOutput")
        dva = nc.dram_tensor("dva", [128, HL * (SP // 128) * 65], f16, kind="ExternalOutput")
        dat2 = nc.dram_tensor("dat2", [65, NQ], f32, kind="ExternalOutput")
        da0 = nc.dram_tensor("da0", [128, NQ], f16, kind="ExternalOutput")
        da1 = nc.dram_tensor("da1", [64, NQ], f16, kind="ExternalOutput")

    EXPF = mybir.ActivationFunctionType.Exp
    qT_r = qT.ap().rearrange("(c p) n -> p c n", p=128)
    kT_r = kT.ap().rearrange("(c p) n -> p c n", p=128)
    vT_r = vT.ap().rearrange("(c p) n -> p c n", p=128)
    wqkv_r = wqkv.ap().rearrange("(c p) n -> p c n", p=128)

    groups = [(j0, min(j0 + 4, NCH)) for j0 in range(0, NCH, 4)]

    with tile.TileContext(nc) as tc:
        with (
            tc.tile_pool(name="const", bufs=1) as cpool,
            tc.tile_pool(name="work", bufs=2) as wpool,
            tc.tile_pool(name="expp", bufs=6) as epool,
        ):
            # ---- input DMAs: critical slices (q1 weights, qT first half,
            # h2 k weights, first kT group) first, then the bulk streams
            w_in = cpool.tile([128, DC, 3 * HWID], f16)
            nc.sync.dma_start(w_in[:, :, 0:256], wqkv_r[:, :, 0:256])
            qT_in = cpool.tile([128, DC, NQ], f16)
            nc.sync.dma_start(qT_in[:, :, 0:512], qT_r[:, :, 0:512])
            kT_in = cpool.tile([128, DC, SP], f16)
            vT_in = cpool.tile([128, DC, SP], f16)
            j0, j1 = groups[0]
            nc.sync.dma_start(kT_in[:, :, j0 * 128:j1 * 128],
                              kT_r[:, :, j0 * 128:j1 * 128])
            nc.sync.dma_start(qT_in[:, :, 512:1024], qT_r[:, :, 512:1024])
            nc.sync.dma_start(w_in[:, :, 256:576], wqkv_r[:, :, 256:576])
            nc.sync.dma_start(vT_in[:, :, j0 * 128:j1 * 128],
                              vT_r[:, :, j0 * 128:j1 * 128])
            msk = cpool.tile([128, NCH], f16)
            nc.sync.dma_start(msk[:], mv.ap().rearrange("(c p) -> p c", p=128))
            for j0, j1 in groups[1:]:
                nc.sync.dma_start(kT_in[:, :, j0 * 128:j1 * 128],
                                  kT_r[:, :, j0 * 128:j1 * 128])
                nc.sync.dma_start(vT_in[:, :, j0 * 128:j1 * 128],
                                  vT_r[:, :, j0 * 128:j1 * 128])
            wo_in = cpool.tile([128, 1536], f16)
            nc.sync.dma_start(wo_in[:], wop.ap())

            # ---- SBUF holders
            q0 = cpool.tile([128, NQ], f16)
            q1 = cpool.tile([64, NQ], f16)
            k0 = cpool.tile([128, SP], f16)
            k1 = cpool.tile([64, SP], f16)
            vaug = cpool.tile([128, HL * NCH * 65], f16)
            vaug_r = vaug[:].rearrange("p (h j e) -> p h j e", h=HL, j=NCH)
            a0 = cpool.tile([128, NQ], f16)
            a1 = cpool.tile([64, NQ], f16)
            at2_sb = cpool.tile([65, NQ], f32)

            # ones column of vaug (gated by mask; zero for padded rows)
            nc.vector.tensor_copy(
                vaug_r[:, :, :, 64],
                msk[:].rearrange("p (u j) -> p u j", u=1).broadcast_to([128, HL, NCH]),
            )

            # softmax denominator: dst = num * (1/z) without touching the
            # Activation engine (DVE fast reciprocal + Pool broadcast).
            def normalize(z_ap, num_ap, dst_ap):
                # custom-DVE ops can't read partition-shifted APs on hw:
                # stage the Z row to partition 0 via an Activation copy first
                zrow = wpool.tile([1, 512], f32, tag="zrow")
                nc.scalar.copy(zrow[:], z_ap)
                rz = wpool.tile([1, 512], f32, tag="rz")
                nc.vector.reciprocal_approx_fast(rz[:], zrow[:])
                rzb = wpool.tile([64, 512], f32, tag="rzb")
                nc.gpsimd.partition_broadcast(rzb[:], rz[:])
                nc.vector.tensor_mul(dst_ap, num_ap, rzb[:])

            # =========== pool A: fill + pass1 (q/k/v proj + h2) ===========
            with tc.tile_pool(name="psA", bufs=1, space="PSUM") as pA:
                def qproj(dst, wc, mw, nf, w=512):
                    ps = pA.tile([128, NQ], f32, tag="sc", bufs=2)
                    for d in range(DC):
                        nc.tensor.matmul(
                            ps[0:mw, nf:nf + w],
                            w_in[:, d, wc:wc + mw],
                            qT_in[:, d, nf:nf + w],
                            start=(d == 0), stop=(d == DC - 1),
                        )
                    nc.vector.tensor_copy(dst[:, nf:nf + w], ps[0:mw, nf:nf + w])

                qproj(q1, 0, 64, 0)           # h2 queries, first half

                at2a = pA.tile([65, 512], f32, tag="at2a")
                at2b = pA.tile([65, 512], f32, tag="at2b")
                # h0's exp tiles are produced in pass1 (Act is otherwise
                # idle there) and consumed by pass2's attn matmuls
                ex0s = [cpool.tile([128, NQ], f16, name=f"ex0_{j}")
                        for j in range(NCH)]

                prev = None
                for gi, (j0, j1) in enumerate(groups):
                    gw = (j1 - j0) * 128
                    # h2 k rows first (pass1 needs them); pair rows after
                    kp2 = pA.tile([128, 512], f32, tag="kp")
                    for d in range(DC):
                        nc.tensor.matmul(
                            kp2[0:64, 0:gw], w_in[:, d, 64:128],
                            kT_in[:, d, j0 * 128:j1 * 128],
                            start=(d == 0), stop=(d == DC - 1),
                        )
                    nc.vector.tensor_copy(k1[:, j0 * 128:j1 * 128], kp2[0:64, 0:gw])
                    if gi == 0:
                        qproj(q1, 0, 64, 512)
                    kp = pA.tile([128, 512], f32, tag="kp")
                    for d in range(DC):
                        nc.tensor.matmul(
                            kp[:, 0:gw], w_in[:, d, 128:256],
                            kT_in[:, d, j0 * 128:j1 * 128],
                            start=(d == 0), stop=(d == DC - 1),
                        )
                    nc.vector.tensor_copy(k0[:, j0 * 128:j1 * 128], kp[:, 0:gw])

                    for j in range(j0, j1):
                        # v projection chunk j (all heads)
                        vp = pA.tile([128, HWID], f32, tag="vp")
                        for d in range(DC):
                            nc.tensor.matmul(
                                vp[:], vT_in[:, d, j * 128:(j + 1) * 128],
                                w_in[:, d, 384:576],
                                start=(d == 0), stop=(d == DC - 1),
                            )
                        nc.vector.tensor_copy(
                            vaug_r[:, :, j, 0:64],
                            vp[:].rearrange("p (h e) -> p h e", h=HL),
                        )
                        # h2 scores chunk j
                        sc = pA.tile([128, NQ], f32, tag="sc", bufs=2)
                        for nf in (0, 512):
                            nc.tensor.matmul(
                                sc[:, nf:nf + 512], k1[:, j * 128:(j + 1) * 128],
                                q1[:, nf:nf + 512], start=True, stop=True,
                            )
                        ex = epool.tile([128, NQ], f16, tag="ex")
                        nc.scalar.activation(ex[:], sc[:], EXPF, scale=SCALE)
                        # q0 projection as PE filler right after the first
                        # h2 scores (needed before the first h0 scores)
                        if gi == 0 and j == j0:
                            qproj(q0, 256, 128, 0)
                            qproj(q0, 256, 128, 512)
                        if prev is not None:
                            pj, pex = prev
                            for nf, att in ((0, at2a), (512, at2b)):
                                nc.tensor.matmul(
                                    att[:, 0:512],
                                    vaug[:, (2 * NCH + pj) * 65:(2 * NCH + pj) * 65 + 65],
                                    pex[:, nf:nf + 512],
                                    start=(pj == 0), stop=False,
                                )
                        # h0 scores + exp (stored for pass2)
                        sch0 = pA.tile([128, NQ], f32, tag="sc", bufs=2)
                        for nf in (0, 512):
                            nc.tensor.matmul(
                                sch0[:, nf:nf + 512], k0[0:64, j * 128:(j + 1) * 128],
                                q0[0:64, nf:nf + 512], start=True, stop=True,
                            )
                        nc.scalar.activation(ex0s[j][:], sch0[:], EXPF, scale=SCALE)
                        prev = (j, ex)
                pj, pex = prev
                # evacuate each h2 accumulator half as soon as it stops so
                # pool A's release isn't gated on one long serial chain
                nc.tensor.matmul(
                    at2a[:, 0:512],
                    vaug[:, (2 * NCH + pj) * 65:(2 * NCH + pj) * 65 + 65],
                    pex[:, 0:512], start=(pj == 0), stop=True,
                )
                nc.vector.tensor_copy(at2_sb[:, 0:512], at2a[:])
                nc.tensor.matmul(
                    at2b[:, 0:512],
                    vaug[:, (2 * NCH + pj) * 65:(2 * NCH + pj) * 65 + 65],
                    pex[:, 512:1024], start=(pj == 0), stop=True,
                )
                nc.vector.tensor_copy(at2_sb[:, 512:1024], at2b[:])

            # =========== pools B: pass2 (h0+h1), N-split halves ===========
            with tc.tile_pool(name="psAt", bufs=1, space="PSUM") as pAt:
                at0a = pAt.tile([65, 512], f32, tag="at0a")
                at0b = pAt.tile([65, 512], f32, tag="at0b")
                at1a = pAt.tile([65, 512], f32, tag="at1a")
                at1b = pAt.tile([65, 512], f32, tag="at1b")
                with (
                    tc.tile_pool(name="psB", bufs=2, space="PSUM") as pB,
                    tc.tile_pool(name="psC", bufs=2, space="PSUM") as pC,
                ):
                    # h2 normalize: DVE+Pool only, overlaps pass2 compute
                    normalize(at2_sb[64:65, 0:512], at2_sb[0:64, 0:512],
                              a1[:, 0:512])
                    normalize(at2_sb[64:65, 512:1024], at2_sb[0:64, 512:1024],
                              a1[:, 512:1024])

                    def wo_nt(nt):
                        # both contraction halves accumulate in PSUM; D is
                        # split 384/384 so a Wo unit only needs 2x1 bank
                        obs = []
                        for h in (0, 1):
                            po = pC.tile([128, 384], f32, tag="po")
                            nc.tensor.matmul(
                                po[:], a1[:, nt * 128:(nt + 1) * 128],
                                wo_in[0:64, 768 + h * 384:768 + h * 384 + 384],
                                start=True, stop=False,
                            )
                            nc.tensor.matmul(
                                po[:], a0[:, nt * 128:(nt + 1) * 128],
                                wo_in[:, h * 384:h * 384 + 384],
                                start=False, stop=True,
                            )
                            obs.append(po)
                        ob = wpool.tile([128, D], f16, tag="ob", bufs=4)
                        nc.vector.tensor_copy(ob[:, 0:384], obs[0][:])
                        nc.scalar.copy(ob[:, 384:768], obs[1][:])
                        nc.sync.dma_start(out[nt * 128:(nt + 1) * 128, :], ob[:])

                    def half_pass(nf, atx, aty, fillers, a0dst):
                        # h0's attn reads pass1's stored exps: emit two per
                        # chunk so atx stops at midpass and h0's normalize
                        # runs as a filler instead of serializing the tail
                        prev = None
                        for j in range(NCH):
                            sc1 = pB.tile([128, 512], f32, tag="sc2")
                            nc.tensor.matmul(
                                sc1[:], k0[64:128, j * 128:(j + 1) * 128],
                                q0[64:128, nf:nf + 512], start=True, stop=True,
                            )
                            ex1 = epool.tile([128, 512], f16, tag="ex")
                            nc.scalar.activation(ex1[:], sc1[:], EXPF, scale=SCALE)
                            nc.tensor.matmul(
                                atx[:, 0:512],
                                vaug[:, (0 * NCH + j) * 65:(0 * NCH + j) * 65 + 65],
                                ex0s[j][:, nf:nf + 512],
                                start=(j == 0), stop=(j == NCH - 1),
                            )
                            if prev is not None:
                                pj, pex1 = prev
                                nc.tensor.matmul(
                                    aty[:, 0:512],
                                    vaug[:, (1 * NCH + pj) * 65:(1 * NCH + pj) * 65 + 65],
                                    pex1[:], start=(pj == 0), stop=False,
                                )
                            if j in fillers:
                                wo_nt(fillers[j])
                            prev = (j, ex1)
                        pj, pex1 = prev
                        nc.tensor.matmul(
                            aty[:, 0:512],
                            vaug[:, (1 * NCH + pj) * 65:(1 * NCH + pj) * 65 + 65],
                            pex1[:], start=(pj == 0), stop=True,
                        )

                    half_pass(0, at0a, at1a, {}, None)
                    normalize(at0a[64:65, :], at0a[0:64, :], a0[0:64, 0:512])
                    normalize(at1a[64:65, :], at1a[0:64, :], a0[64:128, 0:512])
                    # Wo units 0-3 (query cols 0:512) run as pass2b fillers
                    half_pass(512, at0b, at1b, {5: 0, 8: 1, 11: 2, 14: 3}, None)
                    normalize(at0b[64:65, :], at0b[0:64, :], a0[0:64, 512:1024])
                    normalize(at1b[64:65, :], at1b[0:64, :], a0[64:128, 512:1024])
                    for nt in range(4, 8):
                        wo_nt(nt)
    nc.compile()
    return nc


def _get_program(SP: int):
    if SP not in _programs:
        _programs[SP] = _build(SP)
    return _programs[SP]


def prepare(query, key, value, mask, Wq, Wk, Wv, Wo, bo):
    """Host prep: returns (nc, in_maps, assemble) where assemble(results)
    builds the full (B, N, D) output."""
    query = np.asarray(query, np.float32)
    key = np.asarray(key, np.float32)
    value = np.asarray(value, np.float32)
    mask = np.asarray(mask, np.float32)
    Wq = np.asarray(Wq, np.float32)
    Wk = np.asarray(Wk, np.float32)
    Wv = np.asarray(Wv, np.float32)
    Wo = np.asarray(Wo, np.float32)
    bo = np.asarray(bo, np.float32)

    B, N, _ = query.shape
    idxs = [np.nonzero(mask[b] > 0.5)[0] for b in range(B)]
    se_max = max(len(i) for i in idxs)
    SP = max(((se_max + 127) // 128) * 128, 128)
    nc = _get_program(SP)

    in_maps = []
    for c in range(8):
        b, g = c // 4, c % 4
        hs = g * HWID
        idx = idxs[b]
        ne = len(idx)
        kTc = np.zeros((D, SP), np.float16)
        kTc[:, :ne] = key[b].T[:, idx].astype(np.float16)
        vTc = np.zeros((D, SP), np.float16)
        vTc[:, :ne] = value[b].T[:, idx].astype(np.float16)
        mvec = np.zeros((SP,), np.float16)
        mvec[:ne] = 1.0
        wqT = Wq[hs:hs + HWID, :].T
        wkT = Wk[hs:hs + HWID, :].T
        wvT = Wv[hs:hs + HWID, :].T
        wqkv = np.concatenate([
            wqT[:, 128:192], wkT[:, 128:192], wkT[:, 0:128], wqT[:, 0:128], wvT,
        ], axis=1).astype(np.float16)
        woT = Wo[:, hs:hs + HWID].T.astype(np.float16)   # [192, 768]
        wop = np.zeros((128, 1536), np.float16)
        wop[:, 0:768] = woT[0:128]
        wop[0:64, 768:1536] = woT[128:192]
        in_maps.append({
            "qT": np.ascontiguousarray(query[b].T.astype(np.float16)),
            "kT": kTc,
            "vT": vTc,
            "mv": mvec,
            "wqkv": np.ascontiguousarray(wqkv),
            "wop": wop,
        })

    def assemble(res):
        out = np.zeros((B, N, D), np.float32)
        for b in range(B):
            out[b] = res[4 * b]["out"].astype(np.float32) \
                + res[4 * b + 1]["out"].astype(np.float32) \
                + res[4 * b + 2]["out"].astype(np.float32) \
                + res[4 * b + 3]["out"].astype(np.float32) + bo
        return out

    return nc, in_maps, assemble


def kernel(query, key, value, mask, Wq, Wk, Wv, Wo, bo):
    nc, in_maps, assemble = prepare(query, key, value, mask, Wq, Wk, Wv, Wo, bo)
    res = run_bass_kernel_spmd(nc, in_maps, list(range(8))).results
    return assemble(res)
